# revision 48
# baseline (speedup 1.0000x reference)
"""MiniMHSA Trainium2 kernel: 8 NeuronCores, shard = (batch n, head-group).

Reference computes, per batch n:
  qkv = x @ W_qkv.T + b_qkv ; split into q,k,v heads (H=16, HD=64)
  scores = (q @ k.T) / sqrt(HD), masked keys -> -1e9, softmax, @ v
  out = attn_out @ W_out.T + b_out

Core c handles n = c//2 and head-group hg = c%2 (8 heads each).

Key optimization: masked keys (mask=True) contribute exactly zero to the
softmax (exp(-1e9+s) underflows to 0), so the host gathers the unmasked
key positions per batch into a dense xkv [D, Lk] (Lk = padded max count,
~L/2 for this problem's Bernoulli(0.5) mask) and the kernel only computes
k/v projections, scores, exp and PV over those keys. Pad key slots get a
-1e9 exp bias so they vanish identically.

Device-side dataflow (x and qkv weights in bf16, attention in float32r):
  1) kT = Wk.T @ xkv      [512 rows, Lk]   (+ per-partition bias)
     v  = xkv.T @ Wv      [Lk, 512] + ones column (softmax denominators)
  2) qT = Wq.T @ x        [512 rows, L]    (q pre-scaled by 1/sqrt(HD))
  3) per (q-block of 512, head): S^T = k.T @ q -> exp (mask bias kills
     pads) -> O^T accumulated over key chunks; row 64 = denominators.
     The PV matmul lags the score/exp stream by 2 chunks and each head's
     last PVs + reciprocal defer past the next head's first score so the
     Act engine (the 176us exp floor) never drains.
  4) normalize via reciprocal + ones-outer-product broadcast (deferred
     into the next head), writing head-pair-packed otn [128, 4, L] in SBUF
  5) yT = Wo rows @ otn    [D, L] output TRANSPOSED (bf16); head-pair
     packing gives K=128 contraction; b_out added on DVE during psum
     drain. Out-proj and 3/4 of the q-projection are emitted as small
     filler units inside the attention loop so the PE executes them
     during Act-bound stretches.

Host sums the two head-group partials per batch (f32) and transposes.
"""
import sys

sys.path.insert(0, '/opt/trn_rl_repo')

from collections import deque

import numpy as np

_KERNEL_CACHE = {}


def _split_excess_waits(nc):
    """Walrus codegen reliably accepts only ONE sync wait per instruction
    (Matmult hard-fails at 2, Drain at 5). Tile's scheduler can attach more.
    Move excess waits onto preceding same-engine NOPs — semantically identical
    since engine queues execute in order."""
    from concourse import mybir

    for f in nc.m.functions:
        for blk in f.blocks:
            il = blk.instructions
            i = 0
            while i < len(il):
                inst = il[i]
                si = inst.sync_info
                waits = list(si.on_wait) if si is not None and si.on_wait else []
                if len(waits) > 1:
                    keep = waits[-1:]
                    excess = waits[:-1]
                    pos = i
                    # keep Ldweights/Matmult pairs adjacent: the NOP goes
                    # before the Ldweights, not between it and its Matmult
                    if (type(inst).__name__ == "InstMatmult" and pos > 0
                            and type(il[pos - 1]).__name__ == "InstLdweights"
                            and il[pos - 1].engine == inst.engine):
                        pos -= 1
                    for j, wcond in enumerate(excess):
                        nop = mybir.InstNoOp(name=f"{inst.name}-ws{j}", ins=[], outs=[])
                        nop.engine = inst.engine
                        nop.sync_info = mybir.SyncInfo(on_wait=[wcond], on_update=[])
                        il.insert(pos, nop)
                        pos += 1
                        i += 1
                    inst.sync_info = mybir.SyncInfo(
                        on_wait=keep,
                        on_update=list(si.on_update) if si.on_update else [],
                    )
                i += 1


def _make_cfg(x, mask):
    N, L, D = x.shape
    H = 16
    HD = D // H
    n_unmasked = (~np.asarray(mask, bool)).sum(axis=1).max()
    Lk = max(128, int(-(-int(n_unmasked) // 128)) * 128)
    return {"L": L, "D": D, "HC": H // 2, "HD": HD, "Lk": Lk}


def _build(cfg, waitsplit=True, debug_out=None):
    import concourse.bass as bass
    import concourse.tile as tile
    from concourse import mybir

    F32 = mybir.dt.float32
    F32R = mybir.dt.float32r
    AF = mybir.ActivationFunctionType
    MULT = mybir.AluOpType.mult

    L, D, HC, HD = cfg["L"], cfg["D"], cfg["HC"], cfg["HD"]
    Lk = cfg["Lk"]
    DCH = D // 128            # contraction chunks for projections
    DV = HC * HD              # qkv width per core (512)
    MC = DV // 128            # row chunks of q/k (4)
    KCk = Lk // 128           # attention key chunks
    QHB = L // 512            # attention q blocks
    QLC = L // 512            # q-proj column chunks (512 wide)
    PAIRS = HC // 2           # head pairs for out-proj packing
    DOUT = D
    DC = DOUT // 128          # out-proj row chunks (yT partitions)
    # k-proj column chunks: 384-wide (1.5KB psum, full-rate f32r), tail
    # chunk of Lk%384 if any (multiple of 128)
    kcols = [384] * (Lk // 384)
    if Lk % 384:
        kcols.append(Lk % 384)

    nc = bass.Bass()
    BF16 = mybir.dt.bfloat16
    xT_d = nc.dram_tensor("xT", [D, L], BF16, kind="ExternalInput")
    xkv_d = nc.dram_tensor("xkv", [D, Lk], BF16, kind="ExternalInput")
    wq_d = nc.dram_tensor("wq", [128, DCH, DV], BF16, kind="ExternalInput")
    wk_d = nc.dram_tensor("wk", [128, DCH, DV], BF16, kind="ExternalInput")
    wv_d = nc.dram_tensor("wv", [128, DCH, DV], BF16, kind="ExternalInput")
    bqk_d = nc.dram_tensor("bqk", [128, 2 * MC], F32, kind="ExternalInput")
    bv_d = nc.dram_tensor("bv", [1, DV], F32, kind="ExternalInput")
    mb_d = nc.dram_tensor("mb", [128, KCk], F32, kind="ExternalInput")
    wo_d = nc.dram_tensor("wo", [128, PAIRS, DOUT], F32, kind="ExternalInput")
    bo_d = nc.dram_tensor("boT", [128, DC], F32, kind="ExternalInput")
    y_d = nc.dram_tensor("y", [DOUT, L], BF16, kind="ExternalOutput")
    y2_d = nc.dram_tensor("y2", [DOUT, 512], BF16, kind="ExternalOutput")
    otn_d = None
    if debug_out == "otn":
        otn_d = nc.dram_tensor("otn_o", [128, PAIRS, L], F32, kind="ExternalOutput")

    with tile.TileContext(nc) as tc, \
         nc.allow_low_precision(reason="float32r matmuls intended"):
        with tc.tile_pool(name="const", bufs=1) as const, \
             tc.tile_pool(name="big", bufs=1) as big, \
             tc.tile_pool(name="workP", bufs=(6 if Lk <= 1536 else 4)) as workP, \
             tc.tile_pool(name="workS", bufs=2) as workS, \
             tc.tile_pool(name="psA", bufs=1, space="PSUM") as psA, \
             tc.tile_pool(name="psS", bufs=3, space="PSUM") as psS, \
             tc.tile_pool(name="psO", bufs=2, space="PSUM") as psO, \
             tc.tile_pool(name="psY", bufs=2, space="PSUM") as psY:

            # ---- constants ----
            bqk_t = const.tile([128, 2 * MC], F32)
            nc.sync.dma_start(out=bqk_t, in_=bqk_d[:, :])
            mb_t = const.tile([128, KCk], F32)
            nc.sync.dma_start(out=mb_t, in_=mb_d[:, :])
            boT_t = const.tile([128, DC], F32)
            nc.sync.dma_start(out=boT_t, in_=bo_d[:, :])
            bv_r = const.tile([1, DV], F32R)
            ones_f = const.tile([128, 1], F32)
            nc.vector.memset(ones_f, 1.0)
            ones_r = const.tile([1, 128], F32R)
            nc.vector.tensor_copy(out=ones_r, in_=ones_f[0:1, 0:1].broadcast_to([1, 128]))

            # ---- persistent big tiles ----
            qT_r = big.tile([128, MC, L], F32R, tag="qT")
            kT_r = big.tile([128, MC, Lk], F32R, tag="kT")
            vp_r = big.tile([128, KCk, HC, HD + 1], F32R, tag="vp")
            otn_r = big.tile([128, PAIRS, L], F32R, tag="otn")
            y2_sb = big.tile([128, DC, 512], BF16, tag="y2sb")
            nc.vector.tensor_copy(
                out=vp_r[:, :, :, HD:HD + 1],
                in_=ones_f.unsqueeze(1).unsqueeze(1).broadcast_to([128, KCk, HC, 1]),
            )

            xkv_re = xkv_d.rearrange("(c p) l -> p c l", p=128)
            xkv_bufs = 2 if Lk <= 1408 else 1
            import itertools
            _proj_rot = itertools.cycle([(psA, "proj"), (psY, "y"), (psY, "y")])

            def proj_ps():
                pool, tag = next(_proj_rot)
                return pool.tile([128, 512], F32, tag=tag, name="pps")

            # ---- q pools opened early: wq + first x-block prefetch overlap
            # the k/v projection compute ----
            xT_re = xT_d.rearrange("(c p) l -> p c l", p=128)
            _wq_cm = tc.tile_pool(name="wqp", bufs=1)
            _xq_cm = tc.tile_pool(name="xq", bufs=(2 if Lk <= 1536 else 1))
            wqp = _wq_cm.__enter__()
            xqp = _xq_cm.__enter__()
            wq_r = wqp.tile([128, DCH, DV], BF16)
            xq_tiles = {}

            def prefetch_q0():
                # emitted after the first xkv chunk so it doesn't delay the
                # k-projection's first tiles on either DMA ring
                nc.scalar.dma_start(out=wq_r, in_=wq_d[:, :, :])
                xq_t0 = xqp.tile([128, DCH, 512], BF16)
                nc.gpsimd.dma_start(out=xq_t0, in_=xT_re[:, :, 0:512])
                xq_tiles[0] = xq_t0

            # ---- k+v projection (single pass over xkv, bf16 inputs) ----
            with tc.tile_pool(name="wkv", bufs=1) as wkvp, \
                 tc.tile_pool(name="xkv1", bufs=xkv_bufs) as xkvp:
                wk_r = wkvp.tile([128, DCH, DV], BF16, tag="wk")
                nc.scalar.dma_start(out=wk_r[:, 0:DCH // 2, :],
                                    in_=wk_d[:, 0:DCH // 2, :])
                nc.scalar.dma_start(out=wk_r[:, DCH // 2:DCH, :],
                                    in_=wk_d[:, DCH // 2:DCH, :])
                wv_r = wkvp.tile([128, DCH, DV], BF16, tag="wv")
                nc.scalar.dma_start(out=wv_r, in_=wv_d[:, :, :])
                col0 = 0
                for ci, cw in enumerate(kcols):
                    xkv_t = xkvp.tile([128, DCH, 384], BF16)
                    if ci == 0:
                        nc.gpsimd.dma_start(
                            out=xkv_t[:, 0:DCH // 2, 0:cw],
                            in_=xkv_re[:, 0:DCH // 2, col0:col0 + cw])
                        nc.gpsimd.dma_start(
                            out=xkv_t[:, DCH // 2:DCH, 0:cw],
                            in_=xkv_re[:, DCH // 2:DCH, col0:col0 + cw])
                        nc.gpsimd.dma_start(out=bv_r, in_=bv_d[:, :])
                    else:
                        nc.gpsimd.dma_start(
                            out=xkv_t[:, :, 0:cw], in_=xkv_re[:, :, col0:col0 + cw])
                    if ci == 1 or (ci == 0 and len(kcols) == 1):
                        prefetch_q0()
                    for mc in range(MC):
                        ps = proj_ps()
                        for kch in range(DCH):
                            nc.tensor.matmul(
                                ps[:, 0:cw],
                                wk_r[:, kch, mc * 128:(mc + 1) * 128],
                                xkv_t[:, kch, 0:cw],
                                start=(kch == 0), stop=(kch == DCH - 1),
                            )
                        nc.vector.tensor_scalar_add(
                            out=kT_r[:, mc, col0:col0 + cw],
                            in0=ps[:, 0:cw], scalar1=bqk_t[:, MC + mc:MC + mc + 1],
                        )
                    for sub in range(cw // 128):
                        kcg = col0 // 128 + sub
                        ps = proj_ps()
                        for kch in range(DCH):
                            nc.tensor.matmul(
                                ps[:, :],
                                xkv_t[:, kch, sub * 128:(sub + 1) * 128],
                                wv_r[:, kch, :],
                                start=(kch == 0), stop=False,
                            )
                        nc.tensor.matmul(
                            ps[:, :], ones_r[0:1, :], bv_r[0:1, :],
                            start=False, stop=True,
                        )
                        nc.vector.tensor_copy(
                            out=vp_r[:, kcg, :, 0:HD],
                            in_=ps.rearrange("p (h d) -> p h d", h=HC),
                        )
                    col0 += cw

            # out-proj weights live where the k/v pools just freed; the
            # casting DMA (f32->f32r rounding) queues behind earlier loads
            _wop_cm = tc.tile_pool(name="wop", bufs=1)
            wop = _wop_cm.__enter__()
            wo_r = wop.tile([128, PAIRS, DOUT], F32R)
            nc.gpsimd.dma_start(out=wo_r, in_=wo_d[:, :, :])

            # ---- q projection ----
            # lc chunk 0 (block 0's q columns) is emitted serially; the
            # rest become filler units drained inside the attention loop.
            def make_q_quarters(lc, mc, ps_fn=None):
                # 512-wide q chunk, split into 4 filler units of 2 kch each
                cell = {}

                def mk(qi):
                    def emit():
                        if qi == 0 and mc == 0 and lc not in xq_tiles:
                            xq_t = xqp.tile([128, DCH, 512], BF16)
                            nc.gpsimd.dma_start(
                                out=xq_t, in_=xT_re[:, :, lc * 512:(lc + 1) * 512])
                            xq_tiles[lc] = xq_t
                        if qi == 0:
                            if ps_fn:
                                cell["ps"] = ps_fn()
                            else:
                                cell["ps"] = psA.tile(
                                    [128, 512], F32, tag="proj", name="qps")
                        ps = cell["ps"]
                        for kch in range(2 * qi, 2 * qi + 2):
                            nc.tensor.matmul(
                                ps[:, :],
                                wq_r[:, kch, mc * 128:(mc + 1) * 128],
                                xq_tiles[lc][:, kch, :],
                                start=(kch == 0), stop=(kch == DCH - 1),
                            )
                        if qi == 3:
                            nc.vector.tensor_scalar_add(
                                out=qT_r[:, mc, lc * 512:(lc + 1) * 512],
                                in0=ps, scalar1=bqk_t[:, mc:mc + 1],
                            )
                    return emit
                return [mk(qi) for qi in range(4)]

            for lc in range(1):
                for mc in range(MC):
                    for h in make_q_quarters(lc, mc, ps_fn=proj_ps):
                        h()

            if debug_out == "proj_only":
                zz = workS.tile([128, 512], BF16, tag="y")
                nc.vector.memset(zz, 0.0)
                nc.sync.dma_start(out=y_d[0:128, 0:512], in_=zz)

            # ---- attention with out-proj filler units ----
            def make_outproj_halves(qhb, dc, part=None):
                # part=None: all 4 pairs -> y. part="A": pairs 0-1 + bias -> y
                # (only needs heads 0-3, can run mid-block). part="B": pairs
                # 2-3 -> y2 (host adds). Used to shrink the last block's tail.
                cell = {}
                q0 = qhb * 512
                pr = {"A": range(PAIRS // 2), "B": range(PAIRS // 2, PAIRS)}
                prs = list(pr.get(part, range(PAIRS)))

                def h0():
                    ps = psY.tile([128, 512], F32, tag="y", name="yps")
                    cell["ps"] = ps
                    half = prs[:len(prs) // 2] if part is None else prs
                    for i, pair in enumerate(half):
                        nc.tensor.matmul(
                            ps[:, :],
                            wo_r[:, pair, dc * 128:(dc + 1) * 128],
                            otn_r[:, pair, q0:q0 + 512],
                            start=(i == 0), stop=(part is not None and i == len(half) - 1),
                        )
                    if part == "A":
                        y_sb = workS.tile([128, 512], BF16, tag="y")
                        nc.vector.tensor_scalar_add(
                            out=y_sb, in0=ps, scalar1=boT_t[:, dc:dc + 1])
                        nc.sync.dma_start(
                            out=y_d[dc * 128:(dc + 1) * 128, q0:q0 + 512],
                            in_=y_sb)
                    elif part == "B":
                        # Act engine is idle in the tail; drain there, into a
                        # staging tile; every second unit ships a 2-chunk DMA
                        nc.scalar.copy(out=y2_sb[:, dc, :], in_=ps)
                        if dc % 2 == 1:
                            nc.sync.dma_start(
                                out=y2_d.rearrange(
                                    "(c p) l -> p c l", p=128)[:, dc - 1:dc + 1, :],
                                in_=y2_sb[:, dc - 1:dc + 1, :])

                def h1():
                    ps = cell["ps"]
                    for pair in prs[len(prs) // 2:]:
                        nc.tensor.matmul(
                            ps[:, :],
                            wo_r[:, pair, dc * 128:(dc + 1) * 128],
                            otn_r[:, pair, q0:q0 + 512],
                            start=False, stop=(pair == PAIRS - 1),
                        )
                    y_sb = workS.tile([128, 512], BF16, tag="y")
                    nc.vector.tensor_scalar_add(
                        out=y_sb, in0=ps, scalar1=boT_t[:, dc:dc + 1])
                    nc.sync.dma_start(
                        out=y_d[dc * 128:(dc + 1) * 128, q0:q0 + 512], in_=y_sb)
                if part is None:
                    return [h0, h1]
                return [h0]

            fillers = deque()
            for lc in range(1, QLC):
                for mc in range(MC):
                    for h in make_q_quarters(lc, mc):
                        fillers.append(("q", lc, h))
            kcnt = 0
            pending_norm = []
            prev_tail = [None]

            def force_q_units(max_lc):
                # everything block qhb reads must be emitted before its first
                # score matmul (engine queues execute in order)
                remain = deque()
                while fillers:
                    kind, tag, emit = fillers.popleft()
                    if kind == "q" and tag <= max_lc:
                        emit()
                    else:
                        remain.append((kind, tag, emit))
                fillers.clear()
                fillers.extend(remain)

            def emit_norm():
                # Deferred normalize for a finished head: by now its recip
                # (issued right at head end, on DVE) is long done, so the bc
                # broadcast matmul doesn't stall the PE queue. The otn mult
                # reads both operands straight from PSUM.
                ot, recip, base, hch, q0 = pending_norm.pop(0)
                bcp = psS.tile([128, 512], F32, tag="st")
                nc.tensor.matmul(
                    bcp[0:64, :], ones_r[0:1, 0:64], recip[0:1, :],
                    start=True, stop=True,
                )
                bc_sb = workS.tile([64, 512], F32, tag="bc")
                nc.vector.tensor_copy(out=bc_sb, in_=bcp[0:64, :])
                nc.vector.tensor_tensor(
                    out=otn_r[base:base + 64, hch, q0:q0 + 512],
                    in0=ot[0:HD, :], in1=bc_sb, op=MULT,
                )

            for qhb in range(QHB if debug_out != "proj_only" else 0):
                q0 = qhb * 512
                if qhb:
                    force_q_units(qhb)
                for hl in range(HC):
                    base = (hl % 2) * 64
                    hch = hl // 2
                    ot = psO.tile([128, 512], F32, tag="ot")
                    # software-pipeline: PV lags the score/exp stream by 2 key
                    # chunks; the previous head's last PVs + recip are deferred
                    # past this head's first score/exp so Act never drains
                    pTs = {}

                    def emit_pv(kc, ot=ot, hl=hl):
                        nc.tensor.matmul(
                            ot[0:HD + 1, :],
                            vp_r[:, kc, hl, :],
                            pTs.pop(kc),
                            start=(kc == 0), stop=(kc == KCk - 1),
                        )

                    def make_tail(pTs=None):
                        pTs_l, ot_l, hl_l = dict(pTs or {}), ot, hl
                        base_l, hch_l, q0_l = base, hch, q0

                        def tail_a():
                            kc = KCk - 2
                            nc.tensor.matmul(
                                ot_l[0:HD + 1, :], vp_r[:, kc, hl_l, :],
                                pTs_l.pop(kc),
                                start=(kc == 0), stop=False,
                            )

                        def tail_b():
                            kc = KCk - 1
                            nc.tensor.matmul(
                                ot_l[0:HD + 1, :], vp_r[:, kc, hl_l, :],
                                pTs_l.pop(kc),
                                start=False, stop=True,
                            )
                            recip = workS.tile([1, 512], F32R, tag="recip")
                            nc.vector.reciprocal(out=recip, in_=ot_l[HD:HD + 1, :])
                            pending_norm.append((ot_l, recip, base_l, hch_l, q0_l))
                        return [tail_a, tail_b]

                    for kc in range(KCk):
                        st = psS.tile([128, 512], F32, tag="st")
                        nc.tensor.matmul(
                            st,
                            kT_r[base:base + 64, hch, kc * 128:(kc + 1) * 128],
                            qT_r[base:base + 64, hch, q0:q0 + 512],
                            start=True, stop=True,
                        )
                        pT = workP.tile([128, 512], F32R, tag="pT")
                        nc.scalar.activation(
                            out=pT, in_=st, func=AF.Exp,
                            bias=mb_t[:, kc:kc + 1], scale=1.0,
                        )
                        pTs[kc] = pT
                        if KCk < 3:
                            # degenerate tiny-Lk path: no software pipelining
                            emit_pv(kc)
                            if kc == KCk - 1:
                                recip = workS.tile([1, 512], F32R, tag="recip")
                                nc.vector.reciprocal(
                                    out=recip, in_=ot[HD:HD + 1, :])
                                pending_norm.append((ot, recip, base, hch, q0))
                                if pending_norm[:-1]:
                                    emit_norm()
                            continue
                        if kc in (0, 1) and prev_tail and prev_tail[0] is not None:
                            prev_tail[0]()
                            del prev_tail[0]
                            if not prev_tail:
                                prev_tail.append(None)
                        if kc >= 2:
                            emit_pv(kc - 2)
                        if kc == 2 and pending_norm:
                            emit_norm()
                        kcnt += 1
                        if kcnt % 2 == 0 and fillers and not pending_norm:
                            fillers.popleft()[2]()
                    if KCk >= 3:
                        prev_tail[:] = make_tail(pTs)
                    if (qhb == QHB - 1 and QHB > 1 and KCk >= 3
                            and hl == HC // 2):
                        # heads 0..HC/2-1 are normalized by now: the first
                        # half-sum of the last block's out-proj can run as
                        # ordinary filler work instead of in the tail
                        for dc in range(DC):
                            for h in make_outproj_halves(qhb, dc, part="A"):
                                fillers.append(("op", qhb, h))
                if qhb == QHB - 1 and QHB > 1 and KCk >= 3:
                    for dc in range(DC):
                        for h in make_outproj_halves(qhb, dc, part="B"):
                            fillers.append(("op", qhb, h))
                else:
                    for dc in range(DC):
                        for h in make_outproj_halves(qhb, dc):
                            fillers.append(("op", qhb, h))
            for t in prev_tail:
                if t is not None:
                    t()
            prev_tail[:] = [None]
            while pending_norm:
                emit_norm()
            while fillers:
                fillers.popleft()[2]()
            _wop_cm.__exit__(None, None, None)
            _xq_cm.__exit__(None, None, None)
            _wq_cm.__exit__(None, None, None)

            if debug_out == "otn":
                nc.sync.dma_start(out=otn_d[:, :, :], in_=otn_r[:, :, :])

    # split multi-waits (walrus allows 1 sync wait per instruction reliably)
    if waitsplit:
        _split_excess_waits(nc)
    return nc


def _prep_inputs(x, mask, W_qkv, b_qkv, W_out, b_out, cfg):
    """Build the 8 per-core input maps (host-side shuffles, float32)."""
    L, D, HC, HD = cfg["L"], cfg["D"], cfg["HC"], cfg["HD"]
    Lk = cfg["Lk"]
    DV = HC * HD
    MC = DV // 128
    KCk = Lk // 128
    DC = D // 128
    N = x.shape[0]
    scale = 1.0 / np.sqrt(HD)
    Wt = np.ascontiguousarray(W_qkv.T).astype(np.float32)    # [D, 3D]
    WoT = np.ascontiguousarray(W_out.T).astype(np.float32)   # [D, D]
    DCH = D // 128

    import ml_dtypes
    BF16 = ml_dtypes.bfloat16

    def wslice(w):  # [D, DV] -> [128, DCH, DV]
        return np.ascontiguousarray(
            w.reshape(DCH, 128, DV).transpose(1, 0, 2)).astype(BF16)

    per_hg = []
    for hg in range(2):
        qs, ks, vs = hg * DV, D + hg * DV, 2 * D + hg * DV
        wq = wslice(Wt[:, qs:qs + DV] * scale)
        wk = wslice(Wt[:, ks:ks + DV])
        wv = wslice(Wt[:, vs:vs + DV])
        bq = (b_qkv[qs:qs + DV] * scale).reshape(MC, 128)
        bk = b_qkv[ks:ks + DV].reshape(MC, 128)
        bqk = np.ascontiguousarray(np.concatenate([bq, bk], axis=0).T)  # [128, 2MC]
        bv = np.ascontiguousarray(b_qkv[vs:vs + DV][None, :])
        # wo: [128, PAIRS, D] — WoT rows for this head-group, head-pair packed
        wo = np.ascontiguousarray(
            WoT[hg * DV:(hg + 1) * DV, :].reshape(HC // 2, 128, D)
            .transpose(1, 0, 2))
        per_hg.append(dict(wq=wq, wk=wk, wv=wv, bqk=bqk, bv=bv, wo=wo))

    # b_out only on hg=0 cores; partials are summed on host (avoid 2x bias)
    boT_full = np.ascontiguousarray(b_out.reshape(DC, 128).T).astype(np.float32)
    boT_zero = np.zeros_like(boT_full)
    xTs, xkvs, mbs = [], [], []
    for n in range(N):
        xTs.append(np.ascontiguousarray(x[n].T).astype(BF16))
        idx = np.flatnonzero(~np.asarray(mask[n], bool))
        xkv = np.zeros((D, Lk), BF16)
        xkv[:, :len(idx)] = x[n].T[:, idx].astype(BF16)
        xkvs.append(xkv)
        mb = np.full(Lk, -1e9, np.float32)
        mb[:len(idx)] = 0.0
        mbs.append(np.ascontiguousarray(mb.reshape(KCk, 128).T))

    in_maps = []
    for c in range(2 * N):
        n, hg = c // 2, c % 2
        d = dict(per_hg[hg])
        d.update(xT=xTs[n], xkv=xkvs[n], mb=mbs[n],
                 boT=(boT_full if hg == 0 else boT_zero))
        in_maps.append(d)
    return in_maps


def kernel(x, mask, W_qkv, b_qkv, W_out, b_out):
    from concourse.bass_utils import run_bass_kernel_spmd

    x = np.asarray(x, dtype=np.float32)
    mask = np.asarray(mask)
    N, L, D = x.shape
    cfg = _make_cfg(x, mask)

    key = (L, D, cfg["Lk"])
    if key not in _KERNEL_CACHE:
        _KERNEL_CACHE[key] = _build(cfg)
    nc = _KERNEL_CACHE[key]

    in_maps = _prep_inputs(
        x, mask,
        np.asarray(W_qkv, np.float32), np.asarray(b_qkv, np.float32),
        np.asarray(W_out, np.float32), np.asarray(b_out, np.float32), cfg,
    )
    res = run_bass_kernel_spmd(nc, in_maps, list(range(2 * N)))
    out = np.empty((N, L, D), np.float32)
    for n in range(N):
        yt = np.zeros((D, L), np.float32)
        for c in (2 * n, 2 * n + 1):
            yt += np.asarray(res.results[c]["y"], np.float32)
            y2 = res.results[c].get("y2")
            if y2 is not None and L >= 1024 and cfg["Lk"] // 128 >= 3:
                yt[:, L - 512:] += np.asarray(y2, np.float32)
        out[n] = yt.T
    return out
